# revision 3
# baseline (speedup 1.0000x reference)
"""ContrastLoss (InfoNCE-style) Trainium2 kernel, data-parallel over batch on 8 cores.

Math (per sample b):
    s[i,j] = (tmap[b,i,j] . qhat[b]) / ||tmap[b,i,j]||        (qhat = normalized pos_query)
    e = exp(s); num = sum(e * pos_mask); den = num + sum(e * neg_mask)
    li = -log(num / (den + EPS)); loss = mean(li over valid samples)

Device layout per core (4 samples, 4096 cells each, H=256):
  cells on SBUF partitions, H on the free dim (natural/contiguous DMA).
  - dot(t, qhat): DVE tensor_tensor_reduce (fused mult + free-dim reduce)
  - sumsq(t):     ScalarE activation(Square, accum_out) (most tiles; a few on
                  DVE for engine balance)
  - 1/||t||:      exp(-0.5*ln(sumsq)) on ScalarE - single activation table set
  - masked sums:  small DVE tensor_tensor_reduce; 128 partial sums per sample
                  are shipped to the host, which does the final tiny reduction
                  (-log, valid masking, mean over 32 samples).
"""

import numpy as np

import concourse.bacc as bacc
import concourse.tile as tile
from concourse import mybir
from concourse.bass_utils import run_bass_kernel_spmd

N_CORES = 8
B, S, H = 32, 64, 256
BS = B // N_CORES          # samples per core
CELLS = S * S              # 4096 cells per sample
SUBT = 16                  # 128-cell sub-tiles per chunk
CH_CELLS = 128 * SUBT      # 2048 cells per chunk (2 MB fp32)
NCH = BS * CELLS // CH_CELLS  # 8 chunks per core
EPS = 1e-8

# Fraction of sum-of-squares tiles moved from ScalarE to DVE for balance:
# tile index t (0..127) goes to DVE when t % DVE_SSQ_MOD == 1.
DVE_SSQ_MOD = 7

_NC_CACHE = {}


def _build_nc():
    A = mybir.ActivationFunctionType
    OP = mybir.AluOpType
    dt = mybir.dt

    nc = bacc.Bacc(
        "TRN2",
        target_bir_lowering=False,
        debug=False,
        enable_asserts=False,
        num_devices=N_CORES,
    )

    t_in = nc.dram_tensor("t_in", [NCH, 128, SUBT * H], dt.float32, kind="ExternalInput").ap()
    q_in = nc.dram_tensor("q_in", [128, BS * H], dt.float32, kind="ExternalInput").ap()
    pm_in = nc.dram_tensor("pm_in", [128, BS * 32], dt.float32, kind="ExternalInput").ap()
    nm_in = nc.dram_tensor("nm_in", [128, BS * 32], dt.float32, kind="ExternalInput").ap()
    parts = nc.dram_tensor("parts", [128, 2 * BS], dt.float32, kind="ExternalOutput").ap()

    with tile.TileContext(nc) as tc:
        with (
            tc.tile_pool(name="chunks", bufs=3) as chpool,
            tc.tile_pool(name="small", bufs=1) as spool,
            tc.tile_pool(name="stats", bufs=2) as stpool,
        ):
            qsb = spool.tile([128, BS * H], dt.float32, tag="qsb")
            nc.sync.dma_start(out=qsb[:], in_=q_in[:])
            pmsb = spool.tile([128, BS * 32], dt.float32, tag="pmsb")
            nc.sync.dma_start(out=pmsb[:], in_=pm_in[:])
            nmsb = spool.tile([128, BS * 32], dt.float32, tag="nmsb")
            nc.sync.dma_start(out=nmsb[:], in_=nm_in[:])

            npart = spool.tile([128, 2 * BS], dt.float32, tag="npart")
            dve_scr = spool.tile([128, H], dt.float32, tag="dve_scr")
            act_scr = spool.tile([128, H], dt.float32, tag="act_scr")
            msk_scr = spool.tile([128, 32], dt.float32, tag="msk_scr")

            gidx = 0
            for s in range(BS):
                dotb = stpool.tile([128, 2 * SUBT], dt.float32, tag="dotb")
                ssqb = stpool.tile([128, 2 * SUBT], dt.float32, tag="ssqb")
                for cl in range(2):
                    ch = chpool.tile([128, SUBT * H], dt.float32, tag="ch")
                    nc.sync.dma_start(out=ch[:], in_=t_in[2 * s + cl])
                    for t in range(SUBT):
                        sub = ch[:, t * H:(t + 1) * H]
                        col = cl * SUBT + t
                        nc.vector.scalar_tensor_tensor(
                            out=dve_scr[:],
                            in0=sub,
                            scalar=0.0,
                            in1=qsb[:, s * H:(s + 1) * H],
                            op0=OP.bypass,
                            op1=OP.mult,
                            accum_out=dotb[:, col:col + 1],
                        )
                        if gidx % DVE_SSQ_MOD == 1:
                            nc.vector.scalar_tensor_tensor(
                                out=dve_scr[:],
                                in0=sub,
                                scalar=0.0,
                                in1=sub,
                                op0=OP.bypass,
                                op1=OP.mult,
                                accum_out=ssqb[:, col:col + 1],
                            )
                        else:
                            nc.scalar.activation(
                                act_scr[:], sub, A.Square,
                                accum_out=ssqb[:, col:col + 1],
                            )
                        gidx += 1

                # Per-sample epilogue on (128, 32) stat tiles.
                lnb = stpool.tile([128, 2 * SUBT], dt.float32, tag="lnb")
                nc.scalar.activation(lnb[:], ssqb[:], A.Ln)
                invn = stpool.tile([128, 2 * SUBT], dt.float32, tag="invn")
                nc.scalar.activation(invn[:], lnb[:], A.Exp, scale=-0.5)
                sb = stpool.tile([128, 2 * SUBT], dt.float32, tag="sb")
                nc.vector.tensor_mul(sb[:], dotb[:], invn[:])
                eb = stpool.tile([128, 2 * SUBT], dt.float32, tag="eb")
                nc.scalar.activation(eb[:], sb[:], A.Exp)
                nc.vector.scalar_tensor_tensor(
                    out=msk_scr[:], in0=eb[:], scalar=0.0,
                    in1=pmsb[:, s * 32:(s + 1) * 32],
                    op0=OP.bypass, op1=OP.mult,
                    accum_out=npart[:, 2 * s:2 * s + 1],
                )
                nc.vector.scalar_tensor_tensor(
                    out=msk_scr[:], in0=eb[:], scalar=0.0,
                    in1=nmsb[:, s * 32:(s + 1) * 32],
                    op0=OP.bypass, op1=OP.mult,
                    accum_out=npart[:, 2 * s + 1:2 * s + 2],
                )

            nc.sync.dma_start(out=parts[:], in_=npart[:])

    nc.compile()
    return nc


def get_nc():
    if "nc" not in _NC_CACHE:
        _NC_CACHE["nc"] = _build_nc()
    return _NC_CACHE["nc"]


def _permute_mask(m):
    """(BS, S, S) bool -> (128, BS*32) f32 matching the device accum layout.

    Cell i (flat, 0..4095) lives at partition (i % 2048) // 16, column
    16*(i // 2048) + (i % 16)."""
    out = np.empty((128, BS, 32), np.float32)
    for s in range(BS):
        a = m[s].reshape(2, 128, SUBT).astype(np.float32)  # (cl, p, t)
        out[:, s, :] = a.transpose(1, 0, 2).reshape(128, 2 * SUBT)
    return np.ascontiguousarray(out).reshape(128, BS * 32)


def make_in_maps(pos_query, tmap, mask2d_pos, mask2d_neg):
    pq = np.asarray(pos_query, dtype=np.float32)
    tm = np.ascontiguousarray(np.asarray(tmap, dtype=np.float32))
    mp = np.asarray(mask2d_pos).astype(bool)
    mn = np.asarray(mask2d_neg).astype(bool)

    qn = np.sqrt(np.sum(pq * pq, axis=-1, keepdims=True, dtype=np.float32))
    qhat = (pq / (qn + np.float32(EPS))).astype(np.float32)

    in_maps = []
    for c in range(N_CORES):
        sl = slice(c * BS, (c + 1) * BS)
        tshard = np.ascontiguousarray(tm[sl]).reshape(NCH, 128, SUBT * H)
        q_rep = np.ascontiguousarray(
            np.broadcast_to(qhat[sl][None, :, :], (128, BS, H))
        ).reshape(128, BS * H)
        in_maps.append({
            "t_in": tshard,
            "q_in": q_rep,
            "pm_in": _permute_mask(mp[sl]),
            "nm_in": _permute_mask(mn[sl]),
        })
    return in_maps, mp, mn


def finish(parts_per_core, mp, mn):
    """parts_per_core: list of (128, 2*BS) arrays -> scalar loss (np.float32)."""
    num = np.zeros(B, np.float32)
    neg = np.zeros(B, np.float32)
    for c in range(N_CORES):
        p = parts_per_core[c]
        for s in range(BS):
            num[c * BS + s] = p[:, 2 * s].sum(dtype=np.float32)
            neg[c * BS + s] = p[:, 2 * s + 1].sum(dtype=np.float32)
    den = num + neg
    with np.errstate(divide="ignore", invalid="ignore", over="ignore"):
        li = -np.log(num / (den + np.float32(EPS)))
    valid = mp.any(axis=(1, 2)) & mn.any(axis=(1, 2))
    n_valid = max(int(valid.sum()), 1)
    loss = np.where(valid, li, np.float32(0.0)).sum(dtype=np.float32) / np.float32(n_valid)
    return np.asarray(loss, dtype=np.float32)


def kernel(pos_query, tmap, mask2d_pos, mask2d_neg):
    in_maps, mp, mn = make_in_maps(pos_query, tmap, mask2d_pos, mask2d_neg)
    nc = get_nc()
    res = run_bass_kernel_spmd(nc, in_maps, list(range(N_CORES)))
    parts_per_core = [res.results[c]["parts"] for c in range(N_CORES)]
    return finish(parts_per_core, mp, mn)


if __name__ == "__main__":
    # Smoke test with random data (no reference).
    rng = np.random.default_rng(0)
    inputs = {
        "pos_query": rng.standard_normal((B, H), dtype=np.float32),
        "tmap": rng.standard_normal((B, S, S, H), dtype=np.float32),
        "mask2d_pos": rng.random((B, S, S)) < 0.05,
        "mask2d_neg": (rng.random((B, S, S)) >= 0.05) & (rng.random((B, S, S)) < 0.35),
    }
    print(kernel(**inputs))


# revision 7
# speedup vs baseline: 716.1814x; 716.1814x over previous
"""ContrastLoss (InfoNCE-style) Trainium2 kernel, data-parallel over batch on 8 cores.

Math (per sample b):
    s[i,j] = (tmap[b,i,j] . qhat[b]) / ||tmap[b,i,j]||        (qhat = normalized pos_query)
    e = exp(s); num = sum(e * pos_mask); den = num + sum(e * neg_mask)
    li = -log(num / (den + EPS)); loss = mean(li over valid samples)

Device layout per core (4 samples, 4096 cells each, H=256):
  cells on SBUF partitions, H on the free dim (natural/contiguous DMA).
  - dot(t, qhat): DVE tensor_tensor_reduce (fused mult + free-dim reduce)
  - sumsq(t):     ScalarE activation(Square, accum_out) (most tiles; a few on
                  DVE for engine balance)
  - 1/||t||:      exp(-0.5*ln(sumsq)) on ScalarE - single activation table set
  - masked sums:  small DVE tensor_tensor_reduce; 128 partial sums per sample
                  are shipped to the host, which does the final tiny reduction
                  (-log, valid masking, mean over 32 samples).
"""

import numpy as np

import concourse.bacc as bacc
import concourse.tile as tile
from concourse import mybir
from concourse.bass_utils import run_bass_kernel_spmd
from concourse.hw_specs import get_activation_tables as _real_gat

_ACT_SET = "natural_log_exp_and_others"  # contains square, ln, exp


def _patched_gat(arch):
    """Force every activation to resolve to the one set containing all our
    functions (square/ln/exp), avoiding per-sample table-set thrashing
    (~2.7us per reload). Indices into act_info.json are preserved."""
    tabs = _real_gat(arch)
    return {k: (v if k == _ACT_SET else set()) for k, v in tabs.items()}


bacc.get_activation_tables = _patched_gat

N_CORES = 8
B, S, H = 32, 64, 256
BS = B // N_CORES          # samples per core
CELLS = S * S              # 4096 cells per sample
SUBT = 16                  # 128-cell sub-tiles per chunk
CH_CELLS = 128 * SUBT      # 2048 cells per chunk (2 MB fp32)
NCH = BS * CELLS // CH_CELLS  # 8 chunks per core
EPS = 1e-8

# Fraction of sum-of-squares tiles moved from ScalarE to DVE for balance:
# tile index t (0..127) goes to DVE when t % DVE_SSQ_MOD == 1.
DVE_SSQ_MOD = 7

_NC_CACHE = {}


def _build_nc(loop_reps=0):
    """loop_reps=0: straight-line kernel. loop_reps=N>0: wrap the whole body
    in a tc.For_i loop that re-runs it N times (identical data; used only for
    differential wall-clock timing of the device execution)."""
    A = mybir.ActivationFunctionType
    OP = mybir.AluOpType
    dt = mybir.dt

    nc = bacc.Bacc(
        "TRN2",
        target_bir_lowering=False,
        debug=False,
        enable_asserts=False,
        num_devices=N_CORES,
    )

    t_in = nc.dram_tensor("t_in", [NCH, 128, SUBT * H], dt.float32, kind="ExternalInput").ap()
    q_in = nc.dram_tensor("q_in", [128, BS * H], dt.float32, kind="ExternalInput").ap()
    pm_in = nc.dram_tensor("pm_in", [128, BS * 32], dt.float32, kind="ExternalInput").ap()
    nm_in = nc.dram_tensor("nm_in", [128, BS * 32], dt.float32, kind="ExternalInput").ap()
    parts = nc.dram_tensor("parts", [128, 2 * BS], dt.float32, kind="ExternalOutput").ap()

    with tile.TileContext(nc) as tc:
        with (
            tc.tile_pool(name="chunks", bufs=3) as chpool,
            tc.tile_pool(name="small", bufs=1) as spool,
            tc.tile_pool(name="stats", bufs=2) as stpool,
        ):
            qsb = spool.tile([128, BS * H], dt.float32, tag="qsb")
            nc.sync.dma_start(out=qsb[:], in_=q_in[:])
            pmsb = spool.tile([128, BS * 32], dt.float32, tag="pmsb")
            nc.sync.dma_start(out=pmsb[:], in_=pm_in[:])
            nmsb = spool.tile([128, BS * 32], dt.float32, tag="nmsb")
            nc.sync.dma_start(out=nmsb[:], in_=nm_in[:])

            npart = spool.tile([128, 2 * BS], dt.float32, tag="npart")
            dve_scr = spool.tile([128, H], dt.float32, tag="dve_scr")
            act_scr = spool.tile([128, H], dt.float32, tag="act_scr")
            msk_scr = spool.tile([128, 32], dt.float32, tag="msk_scr")

            import contextlib
            loop_cm = tc.For_i(0, loop_reps, 1) if loop_reps else contextlib.nullcontext()
            with loop_cm:
                _emit_body(nc, tc, spool, stpool, chpool,
                           t_in, qsb, pmsb, nmsb, npart,
                           dve_scr, act_scr, msk_scr, A, OP, dt)

            nc.sync.dma_start(out=parts[:], in_=npart[:])

    nc.compile()
    return nc


def _emit_body(nc, tc, spool, stpool, chpool, t_in, qsb, pmsb, nmsb, npart,
               dve_scr, act_scr, msk_scr, A, OP, dt):
    H_ = H
    if True:
            gidx = 0
            for s in range(BS):
                dotb = stpool.tile([128, 2 * SUBT], dt.float32, tag="dotb")
                ssqb = stpool.tile([128, 2 * SUBT], dt.float32, tag="ssqb")
                for cl in range(2):
                    ch = chpool.tile([128, SUBT * H], dt.float32, tag="ch")
                    nc.sync.dma_start(out=ch[:], in_=t_in[2 * s + cl])
                    for t in range(SUBT):
                        sub = ch[:, t * H:(t + 1) * H]
                        col = cl * SUBT + t
                        nc.vector.scalar_tensor_tensor(
                            out=dve_scr[:],
                            in0=sub,
                            scalar=0.0,
                            in1=qsb[:, s * H:(s + 1) * H],
                            op0=OP.bypass,
                            op1=OP.mult,
                            accum_out=dotb[:, col:col + 1],
                        )
                        if gidx % DVE_SSQ_MOD == 1:
                            nc.vector.scalar_tensor_tensor(
                                out=dve_scr[:],
                                in0=sub,
                                scalar=0.0,
                                in1=sub,
                                op0=OP.bypass,
                                op1=OP.mult,
                                accum_out=ssqb[:, col:col + 1],
                            )
                        else:
                            nc.scalar.activation(
                                act_scr[:], sub, A.Square,
                                accum_out=ssqb[:, col:col + 1],
                            )
                        gidx += 1

                # Per-sample epilogue on (128, 32) stat tiles.
                lnb = stpool.tile([128, 2 * SUBT], dt.float32, tag="lnb")
                nc.scalar.activation(lnb[:], ssqb[:], A.Ln)
                invn = stpool.tile([128, 2 * SUBT], dt.float32, tag="invn")
                nc.scalar.activation(invn[:], lnb[:], A.Exp, scale=-0.5)
                sb = stpool.tile([128, 2 * SUBT], dt.float32, tag="sb")
                nc.vector.tensor_mul(sb[:], dotb[:], invn[:])
                eb = stpool.tile([128, 2 * SUBT], dt.float32, tag="eb")
                nc.scalar.activation(eb[:], sb[:], A.Exp)
                nc.vector.scalar_tensor_tensor(
                    out=msk_scr[:], in0=eb[:], scalar=0.0,
                    in1=pmsb[:, s * 32:(s + 1) * 32],
                    op0=OP.bypass, op1=OP.mult,
                    accum_out=npart[:, 2 * s:2 * s + 1],
                )
                nc.vector.scalar_tensor_tensor(
                    out=msk_scr[:], in0=eb[:], scalar=0.0,
                    in1=nmsb[:, s * 32:(s + 1) * 32],
                    op0=OP.bypass, op1=OP.mult,
                    accum_out=npart[:, 2 * s + 1:2 * s + 2],
                )


def get_nc(loop_reps=0):
    key = ("nc", loop_reps)
    if key not in _NC_CACHE:
        _NC_CACHE[key] = _build_nc(loop_reps)
    return _NC_CACHE[key]


def _permute_mask(m):
    """(BS, S, S) bool -> (128, BS*32) f32 matching the device accum layout.

    Cell i (flat, 0..4095) lives at partition (i % 2048) // 16, column
    16*(i // 2048) + (i % 16)."""
    out = np.empty((128, BS, 32), np.float32)
    for s in range(BS):
        a = m[s].reshape(2, 128, SUBT).astype(np.float32)  # (cl, p, t)
        out[:, s, :] = a.transpose(1, 0, 2).reshape(128, 2 * SUBT)
    return np.ascontiguousarray(out).reshape(128, BS * 32)


def make_in_maps(pos_query, tmap, mask2d_pos, mask2d_neg):
    pq = np.asarray(pos_query, dtype=np.float32)
    tm = np.ascontiguousarray(np.asarray(tmap, dtype=np.float32))
    mp = np.asarray(mask2d_pos).astype(bool)
    mn = np.asarray(mask2d_neg).astype(bool)

    qn = np.sqrt(np.sum(pq * pq, axis=-1, keepdims=True, dtype=np.float32))
    qhat = (pq / (qn + np.float32(EPS))).astype(np.float32)

    in_maps = []
    for c in range(N_CORES):
        sl = slice(c * BS, (c + 1) * BS)
        tshard = np.ascontiguousarray(tm[sl]).reshape(NCH, 128, SUBT * H)
        q_rep = np.ascontiguousarray(
            np.broadcast_to(qhat[sl][None, :, :], (128, BS, H))
        ).reshape(128, BS * H)
        in_maps.append({
            "t_in": tshard,
            "q_in": q_rep,
            "pm_in": _permute_mask(mp[sl]),
            "nm_in": _permute_mask(mn[sl]),
        })
    return in_maps, mp, mn


def finish(parts_per_core, mp, mn):
    """parts_per_core: list of (128, 2*BS) arrays -> scalar loss (np.float32)."""
    num = np.zeros(B, np.float32)
    neg = np.zeros(B, np.float32)
    for c in range(N_CORES):
        p = parts_per_core[c]
        for s in range(BS):
            num[c * BS + s] = p[:, 2 * s].sum(dtype=np.float32)
            neg[c * BS + s] = p[:, 2 * s + 1].sum(dtype=np.float32)
    den = num + neg
    with np.errstate(divide="ignore", invalid="ignore", over="ignore"):
        li = -np.log(num / (den + np.float32(EPS)))
    valid = mp.any(axis=(1, 2)) & mn.any(axis=(1, 2))
    n_valid = max(int(valid.sum()), 1)
    loss = np.where(valid, li, np.float32(0.0)).sum(dtype=np.float32) / np.float32(n_valid)
    return np.asarray(loss, dtype=np.float32)


def kernel(pos_query, tmap, mask2d_pos, mask2d_neg):
    in_maps, mp, mn = make_in_maps(pos_query, tmap, mask2d_pos, mask2d_neg)
    nc = get_nc()
    res = run_bass_kernel_spmd(nc, in_maps, list(range(N_CORES)))
    parts_per_core = [res.results[c]["parts"] for c in range(N_CORES)]
    return finish(parts_per_core, mp, mn)


if __name__ == "__main__":
    # Smoke test with random data (no reference).
    rng = np.random.default_rng(0)
    inputs = {
        "pos_query": rng.standard_normal((B, H), dtype=np.float32),
        "tmap": rng.standard_normal((B, S, S, H), dtype=np.float32),
        "mask2d_pos": rng.random((B, S, S)) < 0.05,
        "mask2d_neg": (rng.random((B, S, S)) >= 0.05) & (rng.random((B, S, S)) < 0.35),
    }
    print(kernel(**inputs))
